# revision 31
# baseline (speedup 1.0000x reference)
"""Neural CDE forward pass on 8 Trainium2 NeuronCores (Bass/Tile).

Math (per batch element b):
    z0 = u0 @ Wi + bi                                   [64]
    for t in 0..164:
        h  = relu(z @ W1 + b1)                          [128]
        f  = tanh(h @ W2 + b2)                          [512] -> [64, 8]
        z += einsum('hi,i->h', f, dx_t)                 dx_t = coeffs[t+1]-coeffs[t]
    out_t = z_t @ Wr + br  for every t (166 values)

Numerics/perf model (all hardware-measured on this container):
  - The scan is chaotic: errors amplify ~1.05x/step (~3000x over 165 steps).
  - fp32 matmul: exact-grade (1e-7 rel/step) but 4 cycles/row: 559 ns per
    N=256 matmul including the serialized fused weight load.
  - float32r matmul: operands rounded to ~12 mantissa bits (1.4e-4
    rel/step) but 1 cycle/row: 223 ns per N=256 matmul.  An all-f32r scan
    measures 21.3 absmax final error (vs 1.25 allowed = 2e-2 * 62.5).
  - Hybrid phase split: step-t errors are amplified by ~1.05^(165-t), so
    running fp32 for t < T0 and f32r for t >= T0 gives final error
    ~ 5.5e-2 (fp32 part) + 21.3 * 1.05^-T0.  measured absmax vs T0:
    0.321@72, 0.491@65, 0.627@59 (shipped: rel 1.0e-2, 2x inside the
    gate), 1.363@56 - the growth steepens sharply below T0~58, so 59 is
    the knee.  A numpy simulation of the rounding semantics reproduces
    the measured error and shows (a) the late-phase error is spread
    evenly across the h/W2/g roundings - no selective extra pass pays
    for itself - and (b) ranking all 165 steps by per-step cost
    (injected rounding error x remaining amplification, both simulated)
    selects exactly the contiguous prefix {0..58} as the optimal fp32
    set, and shrinking it to 50 steps triples the error: the contiguous
    T0=59 split is the optimum, not an approximation.
    Measured end-to-end: ~1.0-1.1 ms vs the 1.78 ms fp32 baseline (513-
    scan hardware-loop wall, the cleanest comparable, dropped 2.14 s ->
    1.95 s across the final optimization sequence).  The f32r phase is
    bound by the z->h->f->g->e->z dependency cycle - dominated by the
    ScalarE queue (relu + 4 tanh) feeding the g multiplies - so the late
    phase writes tanh output in fp16 (halves the ACT write bytes; f only
    feeds the g multiply, whose result is f32r-rounded to 12 bits anyway,
    so the precision cost is ~5%).  The fp32 phase is PE-capacity-bound.

Kernel design (per core, batch shard B=512 split into NCHAIN=2 chains of
Bc=256 on the matmul free dim):
  - State zT [64+1, Bc] fp32 in SBUF per chain; row 64 carries the running
    readout out_t = z_t @ Wr + br.  mm1 (z -> h pre-act) is always fp32 so
    the state stream never loses precision.
  - h: ScalarE relu with fused per-partition bias b1 -> h tile (fp32 in
    the fp32 phase; declared f32r in the f32r phase - the PE rounds f32r
    operands internally, so no separate rounding op is needed).
  - f: 4 banks of W2.T h (single matmul per bank; fp32 or f32r stationary
    w2_sb / w2_13), tanh with bias b2_j on ScalarE -> f (fp32 early,
    fp16 late).
  - einsum: g_j = f_j * dx_rep on VectorE (fp32 or f32r out);
    dx_rep[p, b] = dx[b, p % 8] is DMA'd per step as [8, B] from HBM and
    partition-replicated 16x by the DMA itself (stride-0 source AP).
    e = sum_j S_j'.T g_j: 4 accumulating matmuls; S_j' [128, 65] has 0/1
    entries (exact in f32r) plus column 64 = S_j @ Wr whose rounding only
    touches the readout, so e[64] = Wr . e_z rides along for free.
  - z_new = z_old + e (VectorE fp32 add); row 64 is DMA'd per step.
  - Pipeline / tiling: the two chains share every stationary operand and
    every per-bank bias, so all wide ops fuse across chains: the two relus
    write halves of ONE [128, 2, Bc] h tile; mm2 is ONE N=512 matmul per
    bank; tanh and the g multiply are ONE [128, 512] op per bank; the
    reduce is ONE N=512 matmul per bank into a shared [65, 2, Bc] e_ps
    whose halves feed the per-chain z adds.  10 PE instructions and 12
    elementwise instructions per step (vs 18+18 in the per-chain form),
    which matters because every matmul carries ~130 ns of fixed
    weight-load/dispatch cost and every engine op ~100-200 ns.  Only mm1,
    relu and the z add stay per-chain: they gate the recurrence cycle, and
    splitting them keeps chain c1's state update off chain c0's critical
    path.  Values are bitwise identical to the per-chain emission.
"""

import numpy as np

IN_CH = 8
HID = 64
MLP_W = 128
OUT = 1
B_FULL, T = 4096, 166
NSTEP = T - 1
N_CORES = 8
B = B_FULL // N_CORES  # 512
NBANK = 4

# engine assignment knobs ("vector" | "gpsimd")
ENG_GMUL = ("vector", "gpsimd", "vector", "gpsimd")
ENG_G13 = ("gpsimd", "vector", "gpsimd", "vector")
ENG_DG = ("vector", "gpsimd", "vector", "gpsimd")
ADD_ON = "vector"
T0 = 59  # steps < T0 run fp32 matmuls; steps >= T0 run f32r
NCHAIN = 2
Bc = B // NCHAIN  # 256
DX_PREFETCH = 3
REPEAT = 1

_CACHE = {}


def _build_bass(repeat=1, knobs=None):
    from contextlib import ExitStack

    import concourse.tile as tile
    from concourse import bacc, mybir

    kn = dict(gmul=ENG_GMUL, g13=ENG_G13, dg=ENG_DG, add=ADD_ON, t0=T0)
    if knobs:
        kn.update(knobs)

    f32 = mybir.dt.float32
    f32r = mybir.dt.float32r
    f16 = mybir.dt.float16
    AF = mybir.ActivationFunctionType
    ALU = mybir.AluOpType

    nc = bacc.Bacc("TRN2", target_bir_lowering=False, debug=False)

    u0t = nc.dram_tensor("u0t", [IN_CH, B], f32, kind="ExternalInput")
    dxt = nc.dram_tensor("dxt", [NSTEP, IN_CH, B], f32, kind="ExternalInput")
    w1 = nc.dram_tensor("w1", [HID, MLP_W], f32, kind="ExternalInput")
    b1 = nc.dram_tensor("b1", [MLP_W, 1], f32, kind="ExternalInput")
    w2 = nc.dram_tensor("w2", [MLP_W, NBANK, 128], f32, kind="ExternalInput")
    b2 = nc.dram_tensor("b2", [128, NBANK], f32, kind="ExternalInput")
    wi = nc.dram_tensor("wi", [IN_CH, HID + 1], f32, kind="ExternalInput")
    smat = nc.dram_tensor("smat", [128, NBANK, HID + 1], f32,
                          kind="ExternalInput")
    outp = nc.dram_tensor("outp", [T, B], f32, kind="ExternalOutput")

    with tile.TileContext(nc) as tc, ExitStack() as ctx:
        const = ctx.enter_context(tc.tile_pool(name="const", bufs=1))
        zpool = ctx.enter_context(tc.tile_pool(name="zpool", bufs=2))
        hpool = ctx.enter_context(tc.tile_pool(name="hpool", bufs=2))
        fpool = ctx.enter_context(tc.tile_pool(name="fpool", bufs=2))
        gpool = ctx.enter_context(tc.tile_pool(name="gpool", bufs=3))
        dxpool = ctx.enter_context(tc.tile_pool(name="dxpool", bufs=5))
        psum_h = ctx.enter_context(tc.tile_pool(name="psum_h", bufs=2, space="PSUM"))
        psum_f = ctx.enter_context(tc.tile_pool(name="psum_f", bufs=2, space="PSUM"))
        psum_e = ctx.enter_context(tc.tile_pool(name="psum_e", bufs=2, space="PSUM"))

        w1_sb = const.tile([HID, MLP_W], f32)
        nc.sync.dma_start(w1_sb[:], w1[:])
        b1_sb = const.tile([MLP_W, 1], f32)
        nc.sync.dma_start(b1_sb[:], b1[:])
        w2_sb = const.tile([MLP_W, NBANK, 128], f32)
        nc.sync.dma_start(w2_sb[:], w2[:])
        b2_sb = const.tile([128, NBANK], f32)
        nc.sync.dma_start(b2_sb[:], b2[:])
        wi_sb = const.tile([IN_CH, HID + 1], f32)
        nc.sync.dma_start(wi_sb[:], wi[:])
        s_sb = const.tile([128, NBANK, HID + 1], f32)
        nc.sync.dma_start(s_sb[:], smat[:])
        s_sb_r = const.tile([128, NBANK, HID + 1], f32r, name="s_sb_r")
        nc.vector.tensor_copy(s_sb_r[:], s_sb[:])
        u0t_sb = const.tile([IN_CH, B], f32)
        nc.sync.dma_start(u0t_sb[:], u0t[:])

        # f32r alias of W2 for the late phase (PE rounds internally; the
        # DVE copy applies the same rounding, value-identical)
        w2_13 = const.tile([MLP_W, NBANK, 128], f32r, name="w2_13")
        nc.vector.tensor_copy(w2_13[:], w2_sb[:])

        z_sb = [None] * NCHAIN
        dx_tiles = {}
        g_banks = [None] * NBANK

        def init_chains():
            z0_ps = psum_e.tile([HID + 1, NCHAIN, Bc], f32, tag="e_ps",
                                name="z0_ps")
            for c in range(NCHAIN):
                cs = slice(c * Bc, (c + 1) * Bc)
                nc.tensor.matmul(
                    z0_ps[:, c, :], wi_sb[:], u0t_sb[:, cs],
                    start=True, stop=True
                )
                z_c = zpool.tile([HID + 1, Bc], f32, tag=f"z{c}", name=f"z_sb{c}")
                nc.vector.tensor_copy(z_c[:], z0_ps[:, c, :])
                nc.sync.dma_start(outp[0:1, cs], z_c[HID : HID + 1, :])
                z_sb[c] = z_c

        def frag_mm1_h(c, t, lo, h_tile):
            """fp32 mm1 per chain; relu+bias (DVE) writes this chain's half
            of the shared h tile."""
            h_ps = psum_h.tile([MLP_W, Bc], f32, tag="h_ps", name="h_ps")
            nc.tensor.matmul(
                h_ps[:], w1_sb[:], z_sb[c][0:HID, :], start=True, stop=True
            )
            # ScalarE: closer to PSUM, and its queue is idle here (the DVE
            # queue still holds the z adds that gate this step's mm1s)
            nc.scalar.activation(
                h_tile[:, c, :], h_ps[:], AF.Relu, bias=b1_sb[:, 0:1]
            )

        def frag_mm2_g(t, h_tile, lo):
            """per bank: both chains' matmuls into one PSUM tile, then ONE
            [128, 512] tanh (b2_j is per-bank, same for both chains) and
            ONE [128, 512] g multiply against the full dx tile."""
            dx_sb = dx_tiles[t]
            w2_use = w2_13 if lo else w2_sb
            for j in range(NBANK):
                f_ps = psum_f.tile([128, NCHAIN, Bc], f32, tag=f"f_ps{j}",
                                   bufs=1, name=f"f_ps{j}")
                nc.tensor.matmul(f_ps[:], w2_use[:, j, :], h_tile[:],
                                 start=True, stop=True)
                f_sb = fpool.tile([128, NCHAIN, Bc], f16 if lo else f32,
                                  tag=f"f_r{j}" if lo else f"f_f{j}",
                                  name=f"f_sb{j}")
                nc.scalar.activation(
                    f_sb[:], f_ps[:], AF.Tanh, bias=b2_sb[:, j : j + 1]
                )
                g_sb = gpool.tile([128, NCHAIN, Bc], f32r if lo else f32,
                                  tag=f"g_r{j}" if lo else f"g_f{j}",
                                  name=f"g_sb{j}")
                getattr(nc, kn["gmul"][j]).tensor_mul(g_sb[:], f_sb[:], dx_sb[:])
                g_banks[j] = g_sb

        def frag_red_both(t, lo):
            e_ps = psum_e.tile([HID + 1, NCHAIN, Bc], f32, tag="e_ps",
                               name="e_ps")
            s_use = s_sb_r if lo else s_sb
            for j in range(NBANK):
                nc.tensor.matmul(e_ps[:], s_use[:, j, :], g_banks[j][:],
                                 start=j == 0, stop=j == NBANK - 1)
            for c in range(NCHAIN):
                cs = slice(c * Bc, (c + 1) * Bc)
                z_prev = z_sb[c]
                z_sb[c] = zpool.tile([HID + 1, Bc], f32, tag=f"z{c}",
                                     name=f"z_sb{c}")
                getattr(nc, kn["add"]).tensor_add(
                    z_sb[c][:], e_ps[:, c, :], z_prev[:]
                )
                nc.sync.dma_start(outp[t + 1 : t + 2, cs],
                                  z_sb[c][HID : HID + 1, :])

        def dma_dx(t):
            if t >= NSTEP:
                return
            dx_sb = dxpool.tile([128, B], f32, tag="dx", name="dx_sb")
            nc.sync.dma_start(
                dx_sb[:], dxt[t][None, :, :].to_broadcast([128 // IN_CH, IN_CH, B])
            )
            dx_tiles[t] = dx_sb
            if t - DX_PREFETCH - 1 in dx_tiles:
                del dx_tiles[t - DX_PREFETCH - 1]

        def scan_body():
            init_chains()
            dx_tiles.clear()
            for t in range(DX_PREFETCH):
                dma_dx(t)
            for t in range(NSTEP):
                lo = t >= kn["t0"]
                dma_dx(t + DX_PREFETCH)
                h_tile = hpool.tile([MLP_W, NCHAIN, Bc], f32r if lo else f32,
                                    tag="h_r" if lo else "h_f", name="h_tile")
                frag_mm1_h(0, t, lo, h_tile)
                frag_mm1_h(1, t, lo, h_tile)
                frag_mm2_g(t, h_tile, lo)
                frag_red_both(t, lo)

        if repeat == 1:
            scan_body()
        else:
            # hardware loop: trip count is a runtime scalar, so timing
            # amplification costs no extra instructions
            with tc.For_i(0, repeat):
                scan_body()

    nc.compile()
    return nc


def _prep_host(u0, coeffs, W1, b1, W2, b2, Wi, bi, Wr, br):
    f32 = np.float32

    u0t_full = np.empty((IN_CH, B_FULL), f32)
    u0t_full[: IN_CH - 1] = u0.T
    u0t_full[IN_CH - 1] = 1.0

    dX = (coeffs[:, 1:] - coeffs[:, :-1]).astype(f32)  # [B_FULL, NSTEP, IN_CH]
    dxt_small = np.ascontiguousarray(dX.transpose(1, 2, 0))  # [NSTEP, 8, B_FULL]

    wi_mat = np.empty((IN_CH, HID + 1), f32)
    wi_mat[: IN_CH - 1, :HID] = Wi
    wi_mat[IN_CH - 1, :HID] = bi
    wi_mat[: IN_CH - 1, HID] = (Wi @ Wr)[:, 0]
    wi_mat[IN_CH - 1, HID] = float(bi @ Wr[:, 0] + br[0])

    w2_banks = np.ascontiguousarray(W2.reshape(MLP_W, NBANK, 128))
    b2_banks = np.ascontiguousarray(b2.reshape(NBANK, 128).T)

    p = np.arange(128)
    s_full = np.zeros((128, NBANK, HID + 1), f32)
    for j in range(NBANK):
        s_full[p, j, 16 * j + p // IN_CH] = 1.0
        s_full[p, j, HID] = Wr[16 * j + p // IN_CH, 0]

    return {
        "u0t": u0t_full,
        "dxt": dxt_small,
        "w1": np.ascontiguousarray(W1.astype(f32)),
        "b1": np.ascontiguousarray(b1.astype(f32).reshape(MLP_W, 1)),
        "w2": w2_banks.astype(f32),
        "b2": b2_banks.astype(f32),
        "wi": wi_mat,
        "smat": s_full,
    }


def _make_in_maps(full):
    in_maps = []
    for c in range(N_CORES):
        sl = slice(c * B, (c + 1) * B)
        in_maps.append(
            {
                "u0t": np.ascontiguousarray(full["u0t"][:, sl]),
                "dxt": np.ascontiguousarray(full["dxt"][:, :, sl]),
                "w1": full["w1"],
                "b1": full["b1"],
                "w2": full["w2"],
                "b2": full["b2"],
                "wi": full["wi"],
                "smat": full["smat"],
            }
        )
    return in_maps


def kernel(u0, coeffs, W1, b1, W2, b2, Wi, bi, Wr, br, repeat=None, knobs=None):
    from concourse.bass_utils import run_bass_kernel_spmd

    full = _prep_host(
        np.asarray(u0, np.float32), np.asarray(coeffs, np.float32),
        np.asarray(W1, np.float32), np.asarray(b1, np.float32),
        np.asarray(W2, np.float32), np.asarray(b2, np.float32),
        np.asarray(Wi, np.float32), np.asarray(bi, np.float32),
        np.asarray(Wr, np.float32).reshape(HID, OUT),
        np.asarray(br, np.float32).reshape(OUT),
    )
    in_maps = _make_in_maps(full)

    rep = REPEAT if repeat is None else repeat
    key = ("nc", rep, tuple(sorted(knobs.items())) if knobs else None)
    if key not in _CACHE:
        _CACHE[key] = _build_bass(rep, knobs)
    nc = _CACHE[key]

    res = run_bass_kernel_spmd(nc, in_maps, core_ids=list(range(N_CORES)))
    outs = res.results

    out_full = np.empty((B_FULL, T, OUT), np.float32)
    for c in range(N_CORES):
        out_full[c * B : (c + 1) * B, :, 0] = outs[c]["outp"].T
    return out_full


# revision 32
# speedup vs baseline: 1.0021x; 1.0021x over previous
"""Neural CDE forward pass on 8 Trainium2 NeuronCores (Bass/Tile).

Math (per batch element b):
    z0 = u0 @ Wi + bi                                   [64]
    for t in 0..164:
        h  = relu(z @ W1 + b1)                          [128]
        f  = tanh(h @ W2 + b2)                          [512] -> [64, 8]
        z += einsum('hi,i->h', f, dx_t)                 dx_t = coeffs[t+1]-coeffs[t]
    out_t = z_t @ Wr + br  for every t (166 values)

Numerics/perf model (all hardware-measured on this container):
  - The scan is chaotic: errors amplify ~1.05x/step (~3000x over 165 steps).
  - fp32 matmul: exact-grade (1e-7 rel/step) but 4 cycles/row: 559 ns per
    N=256 matmul including the serialized fused weight load.
  - float32r matmul: operands rounded to ~12 mantissa bits (1.4e-4
    rel/step) but 1 cycle/row: 223 ns per N=256 matmul.  An all-f32r scan
    measures 21.3 absmax final error (vs 1.25 allowed = 2e-2 * 62.5).
  - Hybrid phase split: step-t errors are amplified by ~1.05^(165-t), so
    running fp32 for t < T0 and f32r for t >= T0 gives final error
    ~ 5.5e-2 (fp32 part) + 21.3 * 1.05^-T0.  measured absmax vs T0:
    0.321@72, 0.491@65, 0.627@59 (shipped: rel 1.0e-2, 2x inside the
    gate), 1.363@56 - the growth steepens sharply below T0~58, so 59 is
    the knee.  A numpy simulation of the rounding semantics reproduces
    the measured error and shows (a) the late-phase error is spread
    evenly across the h/W2/g roundings - no selective extra pass pays
    for itself - and (b) ranking all 165 steps by per-step cost
    (injected rounding error x remaining amplification, both simulated)
    selects exactly the contiguous prefix {0..58} as the optimal fp32
    set, and shrinking it to 50 steps triples the error: the contiguous
    T0=59 split is the optimum, not an approximation.  Rounding the
    late-phase mm1 input (f32r z via a twin add) measures x1.87 error -
    rel 1.96e-2, too close to the gate - so mm1 stays fp32 in both
    phases: the state stream is the most amplification-sensitive input.
    Measured end-to-end: ~1.0-1.1 ms vs the 1.78 ms fp32 baseline (513-
    scan hardware-loop wall, the cleanest comparable, dropped 2.14 s ->
    1.95 s across the final optimization sequence).  The f32r phase is
    bound by the z->h->f->g->e->z dependency cycle - dominated by the
    ScalarE queue (relu + 4 tanh) feeding the g multiplies - so the late
    phase writes tanh output in fp16 (halves the ACT write bytes; f only
    feeds the g multiply, whose result is f32r-rounded to 12 bits anyway,
    so the precision cost is ~5%).  The fp32 phase is PE-capacity-bound.

Kernel design (per core, batch shard B=512 split into NCHAIN=2 chains of
Bc=256 on the matmul free dim):
  - State zT [64+1, Bc] fp32 in SBUF per chain; row 64 carries the running
    readout out_t = z_t @ Wr + br.  mm1 (z -> h pre-act) is always fp32 so
    the state stream never loses precision.
  - h: ScalarE relu with fused per-partition bias b1 -> h tile (fp32 in
    the fp32 phase; declared f32r in the f32r phase - the PE rounds f32r
    operands internally, so no separate rounding op is needed).
  - f: 4 banks of W2.T h (single matmul per bank; fp32 or f32r stationary
    w2_sb / w2_13), tanh with bias b2_j on ScalarE -> f (fp32 early,
    fp16 late).
  - einsum: g_j = f_j * dx_rep on VectorE (fp32 or f32r out);
    dx_rep[p, b] = dx[b, p % 8] is DMA'd per step as [8, B] from HBM and
    partition-replicated 16x by the DMA itself (stride-0 source AP).
    e = sum_j S_j'.T g_j: 4 accumulating matmuls; S_j' [128, 65] has 0/1
    entries (exact in f32r) plus column 64 = S_j @ Wr whose rounding only
    touches the readout, so e[64] = Wr . e_z rides along for free.
  - z_new = z_old + e (VectorE fp32 add); row 64 is DMA'd per step.
  - Pipeline / tiling: the two chains share every stationary operand and
    every per-bank bias, so all wide ops fuse across chains: the two relus
    write halves of ONE [128, 2, Bc] h tile; mm2 is ONE N=512 matmul per
    bank; tanh and the g multiply are ONE [128, 512] op per bank; the
    reduce is ONE N=512 matmul per bank into a shared [65, 2, Bc] e_ps
    whose halves feed the per-chain z adds.  10 PE instructions and 12
    elementwise instructions per step (vs 18+18 in the per-chain form),
    which matters because every matmul carries ~130 ns of fixed
    weight-load/dispatch cost and every engine op ~100-200 ns.  Only mm1,
    relu and the z add stay per-chain: they gate the recurrence cycle, and
    splitting them keeps chain c1's state update off chain c0's critical
    path.  Values are bitwise identical to the per-chain emission.
"""

import numpy as np

IN_CH = 8
HID = 64
MLP_W = 128
OUT = 1
B_FULL, T = 4096, 166
NSTEP = T - 1
N_CORES = 8
B = B_FULL // N_CORES  # 512
NBANK = 4

# engine assignment knobs ("vector" | "gpsimd")
ENG_GMUL = ("vector", "gpsimd", "vector", "gpsimd")
ENG_G13 = ("gpsimd", "vector", "gpsimd", "vector")
ENG_DG = ("vector", "gpsimd", "vector", "gpsimd")
ADD_ON = "vector"
T0 = 59  # steps < T0 run fp32 matmuls; steps >= T0 run f32r
NCHAIN = 2
Bc = B // NCHAIN  # 256
DX_PREFETCH = 3
REPEAT = 1

_CACHE = {}


def _build_bass(repeat=1, knobs=None):
    from contextlib import ExitStack

    import concourse.tile as tile
    from concourse import bacc, mybir

    kn = dict(gmul=ENG_GMUL, g13=ENG_G13, dg=ENG_DG, add=ADD_ON, t0=T0)
    if knobs:
        kn.update(knobs)

    f32 = mybir.dt.float32
    f32r = mybir.dt.float32r
    f16 = mybir.dt.float16
    AF = mybir.ActivationFunctionType
    ALU = mybir.AluOpType

    nc = bacc.Bacc("TRN2", target_bir_lowering=False, debug=False)

    u0t = nc.dram_tensor("u0t", [IN_CH, B], f32, kind="ExternalInput")
    dxt = nc.dram_tensor("dxt", [NSTEP, IN_CH, B], f32, kind="ExternalInput")
    w1 = nc.dram_tensor("w1", [HID, MLP_W], f32, kind="ExternalInput")
    b1 = nc.dram_tensor("b1", [MLP_W, 1], f32, kind="ExternalInput")
    w2 = nc.dram_tensor("w2", [MLP_W, NBANK, 128], f32, kind="ExternalInput")
    b2 = nc.dram_tensor("b2", [128, NBANK], f32, kind="ExternalInput")
    wi = nc.dram_tensor("wi", [IN_CH, HID + 1], f32, kind="ExternalInput")
    smat = nc.dram_tensor("smat", [128, NBANK, HID + 1], f32,
                          kind="ExternalInput")
    outp = nc.dram_tensor("outp", [T, B], f32, kind="ExternalOutput")

    with tile.TileContext(nc) as tc, ExitStack() as ctx:
        const = ctx.enter_context(tc.tile_pool(name="const", bufs=1))
        zpool = ctx.enter_context(tc.tile_pool(name="zpool", bufs=2))
        hpool = ctx.enter_context(tc.tile_pool(name="hpool", bufs=2))
        fpool = ctx.enter_context(tc.tile_pool(name="fpool", bufs=2))
        gpool = ctx.enter_context(tc.tile_pool(name="gpool", bufs=3))
        dxpool = ctx.enter_context(tc.tile_pool(name="dxpool", bufs=5))
        psum_h = ctx.enter_context(tc.tile_pool(name="psum_h", bufs=2, space="PSUM"))
        psum_f = ctx.enter_context(tc.tile_pool(name="psum_f", bufs=2, space="PSUM"))
        psum_e = ctx.enter_context(tc.tile_pool(name="psum_e", bufs=2, space="PSUM"))

        w1_sb = const.tile([HID, MLP_W], f32)
        nc.sync.dma_start(w1_sb[:], w1[:])
        b1_sb = const.tile([MLP_W, 1], f32)
        nc.sync.dma_start(b1_sb[:], b1[:])
        w2_sb = const.tile([MLP_W, NBANK, 128], f32)
        nc.sync.dma_start(w2_sb[:], w2[:])
        b2_sb = const.tile([128, NBANK], f32)
        nc.sync.dma_start(b2_sb[:], b2[:])
        wi_sb = const.tile([IN_CH, HID + 1], f32)
        nc.sync.dma_start(wi_sb[:], wi[:])
        s_sb = const.tile([128, NBANK, HID + 1], f32)
        nc.sync.dma_start(s_sb[:], smat[:])
        s_sb_r = const.tile([128, NBANK, HID + 1], f32r, name="s_sb_r")
        nc.vector.tensor_copy(s_sb_r[:], s_sb[:])
        u0t_sb = const.tile([IN_CH, B], f32)
        nc.sync.dma_start(u0t_sb[:], u0t[:])

        # f32r alias of W2 for the late phase (PE rounds internally; the
        # DVE copy applies the same rounding, value-identical)
        w2_13 = const.tile([MLP_W, NBANK, 128], f32r, name="w2_13")
        nc.vector.tensor_copy(w2_13[:], w2_sb[:])

        z_sb = [None] * NCHAIN
        dx_tiles = {}
        g_banks = [None] * NBANK

        def init_chains():
            z0_ps = psum_e.tile([HID + 1, NCHAIN, Bc], f32, tag="e_ps",
                                name="z0_ps")
            for c in range(NCHAIN):
                cs = slice(c * Bc, (c + 1) * Bc)
                nc.tensor.matmul(
                    z0_ps[:, c, :], wi_sb[:], u0t_sb[:, cs],
                    start=True, stop=True
                )
                z_c = zpool.tile([HID + 1, Bc], f32, tag=f"z{c}", name=f"z_sb{c}")
                nc.vector.tensor_copy(z_c[:], z0_ps[:, c, :])
                nc.sync.dma_start(outp[0:1, cs], z_c[HID : HID + 1, :])
                z_sb[c] = z_c

        def frag_mm1_h(c, t, lo, h_tile):
            """fp32 mm1 per chain; relu+bias (DVE) writes this chain's half
            of the shared h tile."""
            h_ps = psum_h.tile([MLP_W, Bc], f32, tag="h_ps", name="h_ps")
            nc.tensor.matmul(
                h_ps[:], w1_sb[:], z_sb[c][0:HID, :], start=True, stop=True
            )
            # ScalarE: closer to PSUM, and its queue is idle here (the DVE
            # queue still holds the z adds that gate this step's mm1s)
            nc.scalar.activation(
                h_tile[:, c, :], h_ps[:], AF.Relu, bias=b1_sb[:, 0:1]
            )

        def frag_mm2_g(t, h_tile, lo):
            """per bank: both chains' matmuls into one PSUM tile, then ONE
            [128, 512] tanh (b2_j is per-bank, same for both chains) and
            ONE [128, 512] g multiply against the full dx tile."""
            dx_sb = dx_tiles[t]
            w2_use = w2_13 if lo else w2_sb
            for j in range(NBANK):
                f_ps = psum_f.tile([128, NCHAIN, Bc], f32, tag=f"f_ps{j}",
                                   bufs=1, name=f"f_ps{j}")
                nc.tensor.matmul(f_ps[:], w2_use[:, j, :], h_tile[:],
                                 start=True, stop=True)
                f_sb = fpool.tile([128, NCHAIN, Bc], f16 if lo else f32,
                                  tag=f"f_r{j}" if lo else f"f_f{j}",
                                  name=f"f_sb{j}")
                nc.scalar.activation(
                    f_sb[:], f_ps[:], AF.Tanh, bias=b2_sb[:, j : j + 1]
                )
                g_sb = gpool.tile([128, NCHAIN, Bc], f32r if lo else f32,
                                  tag=f"g_r{j}" if lo else f"g_f{j}",
                                  name=f"g_sb{j}")
                getattr(nc, kn["gmul"][j]).tensor_mul(g_sb[:], f_sb[:], dx_sb[:])
                g_banks[j] = g_sb

        def frag_red_both(t, lo):
            e_ps = psum_e.tile([HID + 1, NCHAIN, Bc], f32, tag="e_ps",
                               name="e_ps")
            s_use = s_sb_r if lo else s_sb
            for j in range(NBANK):
                nc.tensor.matmul(e_ps[:], s_use[:, j, :], g_banks[j][:],
                                 start=j == 0, stop=j == NBANK - 1)
            for c in range(NCHAIN):
                cs = slice(c * Bc, (c + 1) * Bc)
                z_prev = z_sb[c]
                z_sb[c] = zpool.tile([HID + 1, Bc], f32, tag=f"z{c}",
                                     name=f"z_sb{c}")
                getattr(nc, kn["add"]).tensor_add(
                    z_sb[c][:], e_ps[:, c, :], z_prev[:]
                )
                nc.sync.dma_start(outp[t + 1 : t + 2, cs],
                                  z_sb[c][HID : HID + 1, :])

        def dma_dx(t):
            if t >= NSTEP:
                return
            dx_sb = dxpool.tile([128, B], f32, tag="dx", name="dx_sb")
            nc.sync.dma_start(
                dx_sb[:], dxt[t][None, :, :].to_broadcast([128 // IN_CH, IN_CH, B])
            )
            dx_tiles[t] = dx_sb
            if t - DX_PREFETCH - 1 in dx_tiles:
                del dx_tiles[t - DX_PREFETCH - 1]

        def scan_body():
            init_chains()
            dx_tiles.clear()
            for t in range(DX_PREFETCH):
                dma_dx(t)
            for t in range(NSTEP):
                lo = t >= kn["t0"]
                dma_dx(t + DX_PREFETCH)
                h_tile = hpool.tile([MLP_W, NCHAIN, Bc], f32r if lo else f32,
                                    tag="h_r" if lo else "h_f", name="h_tile")
                frag_mm1_h(0, t, lo, h_tile)
                frag_mm1_h(1, t, lo, h_tile)
                frag_mm2_g(t, h_tile, lo)
                frag_red_both(t, lo)

        if repeat == 1:
            scan_body()
        else:
            # hardware loop: trip count is a runtime scalar, so timing
            # amplification costs no extra instructions
            with tc.For_i(0, repeat):
                scan_body()

    nc.compile()
    return nc


def _prep_host(u0, coeffs, W1, b1, W2, b2, Wi, bi, Wr, br):
    f32 = np.float32

    u0t_full = np.empty((IN_CH, B_FULL), f32)
    u0t_full[: IN_CH - 1] = u0.T
    u0t_full[IN_CH - 1] = 1.0

    dX = (coeffs[:, 1:] - coeffs[:, :-1]).astype(f32)  # [B_FULL, NSTEP, IN_CH]
    dxt_small = np.ascontiguousarray(dX.transpose(1, 2, 0))  # [NSTEP, 8, B_FULL]

    wi_mat = np.empty((IN_CH, HID + 1), f32)
    wi_mat[: IN_CH - 1, :HID] = Wi
    wi_mat[IN_CH - 1, :HID] = bi
    wi_mat[: IN_CH - 1, HID] = (Wi @ Wr)[:, 0]
    wi_mat[IN_CH - 1, HID] = float(bi @ Wr[:, 0] + br[0])

    w2_banks = np.ascontiguousarray(W2.reshape(MLP_W, NBANK, 128))
    b2_banks = np.ascontiguousarray(b2.reshape(NBANK, 128).T)

    p = np.arange(128)
    s_full = np.zeros((128, NBANK, HID + 1), f32)
    for j in range(NBANK):
        s_full[p, j, 16 * j + p // IN_CH] = 1.0
        s_full[p, j, HID] = Wr[16 * j + p // IN_CH, 0]

    return {
        "u0t": u0t_full,
        "dxt": dxt_small,
        "w1": np.ascontiguousarray(W1.astype(f32)),
        "b1": np.ascontiguousarray(b1.astype(f32).reshape(MLP_W, 1)),
        "w2": w2_banks.astype(f32),
        "b2": b2_banks.astype(f32),
        "wi": wi_mat,
        "smat": s_full,
    }


def _make_in_maps(full):
    in_maps = []
    for c in range(N_CORES):
        sl = slice(c * B, (c + 1) * B)
        in_maps.append(
            {
                "u0t": np.ascontiguousarray(full["u0t"][:, sl]),
                "dxt": np.ascontiguousarray(full["dxt"][:, :, sl]),
                "w1": full["w1"],
                "b1": full["b1"],
                "w2": full["w2"],
                "b2": full["b2"],
                "wi": full["wi"],
                "smat": full["smat"],
            }
        )
    return in_maps


def kernel(u0, coeffs, W1, b1, W2, b2, Wi, bi, Wr, br, repeat=None, knobs=None):
    from concourse.bass_utils import run_bass_kernel_spmd

    full = _prep_host(
        np.asarray(u0, np.float32), np.asarray(coeffs, np.float32),
        np.asarray(W1, np.float32), np.asarray(b1, np.float32),
        np.asarray(W2, np.float32), np.asarray(b2, np.float32),
        np.asarray(Wi, np.float32), np.asarray(bi, np.float32),
        np.asarray(Wr, np.float32).reshape(HID, OUT),
        np.asarray(br, np.float32).reshape(OUT),
    )
    in_maps = _make_in_maps(full)

    rep = REPEAT if repeat is None else repeat
    key = ("nc", rep, tuple(sorted(knobs.items())) if knobs else None)
    if key not in _CACHE:
        _CACHE[key] = _build_bass(rep, knobs)
    nc = _CACHE[key]

    res = run_bass_kernel_spmd(nc, in_maps, core_ids=list(range(N_CORES)))
    outs = res.results

    out_full = np.empty((B_FULL, T, OUT), np.float32)
    for c in range(N_CORES):
        out_full[c * B : (c + 1) * B, :, 0] = outs[c]["outp"].T
    return out_full


# revision 33
# speedup vs baseline: 1.0460x; 1.0438x over previous
"""Neural CDE forward pass on 8 Trainium2 NeuronCores (Bass/Tile).

Math (per batch element b):
    z0 = u0 @ Wi + bi                                   [64]
    for t in 0..164:
        h  = relu(z @ W1 + b1)                          [128]
        f  = tanh(h @ W2 + b2)                          [512] -> [64, 8]
        z += einsum('hi,i->h', f, dx_t)                 dx_t = coeffs[t+1]-coeffs[t]
    out_t = z_t @ Wr + br  for every t (166 values)

Numerics/perf model (all hardware-measured on this container):
  - The scan is chaotic: errors amplify ~1.05x/step (~3000x over 165 steps).
  - fp32 matmul: exact-grade (1e-7 rel/step) but 4 cycles/row: 559 ns per
    N=256 matmul including the serialized fused weight load.
  - float32r matmul: operands rounded to ~12 mantissa bits (1.4e-4
    rel/step) but 1 cycle/row: 223 ns per N=256 matmul.  An all-f32r scan
    measures 21.3 absmax final error (vs 1.25 allowed = 2e-2 * 62.5).
  - Hybrid phase split: step-t errors are amplified by ~1.05^(165-t), so
    running fp32 for t < T0 and f32r for t >= T0 gives final error
    ~ 5.5e-2 (fp32 part) + 21.3 * 1.05^-T0.  measured absmax vs T0:
    0.321@72, 0.491@65, 0.627@59 (shipped: rel 1.0e-2, 2x inside the
    gate), 1.363@56 - the growth steepens sharply below T0~58, so 59 is
    the knee.  A numpy simulation of the rounding semantics reproduces
    the measured error and shows (a) the late-phase error is spread
    evenly across the h/W2/g roundings - no selective extra pass pays
    for itself - and (b) ranking all 165 steps by per-step cost
    (injected rounding error x remaining amplification, both simulated)
    selects exactly the contiguous prefix {0..58} as the optimal fp32
    set, and shrinking it to 50 steps triples the error: the contiguous
    T0=59 split is the optimum, not an approximation.  Rounding the
    late-phase mm1 input (f32r z via a twin add) measures x1.87 error -
    rel 1.96e-2, too close to the gate - so mm1 stays fp32 in both
    phases: the state stream is the most amplification-sensitive input.
    Single-step f32r flips inside the fp32 prefix were also probed in sim
    (deterministic cancellation exists - flipping step 55 improves the
    error) but at most ~2 steps (~7 us) are exploitable: below noise and
    fragile to sim-vs-HW rounding-phase differences, so not taken.
    Measured end-to-end: ~1.0-1.1 ms vs the 1.78 ms fp32 baseline (513-
    scan hardware-loop wall, the cleanest comparable, dropped 2.14 s ->
    1.95 s across the final optimization sequence).  The f32r phase is
    bound by the z->h->f->g->e->z dependency cycle - dominated by the
    ScalarE queue (relu + 4 tanh) feeding the g multiplies - so the late
    phase writes tanh output in fp16 (halves the ACT write bytes; f only
    feeds the g multiply, whose result is f32r-rounded to 12 bits anyway,
    so the precision cost is ~5%).  The fp32 phase is PE-capacity-bound.

Kernel design (per core, batch shard B=512 split into NCHAIN=2 chains of
Bc=256 on the matmul free dim):
  - State zT [64+1, Bc] fp32 in SBUF per chain; row 64 carries the running
    readout out_t = z_t @ Wr + br.  mm1 (z -> h pre-act) is always fp32 so
    the state stream never loses precision.
  - h: ScalarE relu with fused per-partition bias b1 -> h tile (fp32 in
    the fp32 phase; declared f32r in the f32r phase - the PE rounds f32r
    operands internally, so no separate rounding op is needed).
  - f: 4 banks of W2.T h (single matmul per bank; fp32 or f32r stationary
    w2_sb / w2_13), tanh with bias b2_j on ScalarE -> f (fp32 early,
    fp16 late).
  - einsum: g_j = f_j * dx_rep on VectorE (fp32 or f32r out);
    dx_rep[p, b] = dx[b, p % 8] is DMA'd per step as [8, B] from HBM and
    partition-replicated 16x by the DMA itself (stride-0 source AP).
    e = sum_j S_j'.T g_j: 4 accumulating matmuls; S_j' [128, 65] has 0/1
    entries (exact in f32r) plus column 64 = S_j @ Wr whose rounding only
    touches the readout, so e[64] = Wr . e_z rides along for free.
  - z_new = z_old + e (VectorE fp32 add); row 64 is DMA'd per step.
  - Pipeline / tiling: the two chains share every stationary operand and
    every per-bank bias, so all wide ops fuse across chains: the two relus
    write halves of ONE [128, 2, Bc] h tile; mm2 is ONE N=512 matmul per
    bank; tanh and the g multiply are ONE [128, 512] op per bank; the
    reduce is ONE N=512 matmul per bank into a shared [65, 2, Bc] e_ps
    whose halves feed the per-chain z adds.  10 PE instructions and 12
    elementwise instructions per step (vs 18+18 in the per-chain form),
    which matters because every matmul carries ~130 ns of fixed
    weight-load/dispatch cost and every engine op ~100-200 ns.  Only mm1,
    relu and the z add stay per-chain: they gate the recurrence cycle, and
    splitting them keeps chain c1's state update off chain c0's critical
    path.  Values are bitwise identical to the per-chain emission.
"""

import numpy as np

IN_CH = 8
HID = 64
MLP_W = 128
OUT = 1
B_FULL, T = 4096, 166
NSTEP = T - 1
N_CORES = 8
B = B_FULL // N_CORES  # 512
NBANK = 4

# engine assignment knobs ("vector" | "gpsimd")
ENG_GMUL = ("vector", "gpsimd", "vector", "gpsimd")
ENG_G13 = ("gpsimd", "vector", "gpsimd", "vector")
ENG_DG = ("vector", "gpsimd", "vector", "gpsimd")
ADD_ON = "vector"
T0 = 59  # steps < T0 run fp32 matmuls; steps >= T0 run f32r
NCHAIN = 2
Bc = B // NCHAIN  # 256
DX_PREFETCH = 3
REPEAT = 1

_CACHE = {}


def _build_bass(repeat=1, knobs=None):
    from contextlib import ExitStack

    import concourse.tile as tile
    from concourse import bacc, mybir

    kn = dict(gmul=ENG_GMUL, g13=ENG_G13, dg=ENG_DG, add=ADD_ON, t0=T0)
    if knobs:
        kn.update(knobs)

    f32 = mybir.dt.float32
    f32r = mybir.dt.float32r
    f16 = mybir.dt.float16
    AF = mybir.ActivationFunctionType
    ALU = mybir.AluOpType

    nc = bacc.Bacc("TRN2", target_bir_lowering=False, debug=False)

    u0t = nc.dram_tensor("u0t", [IN_CH, B], f32, kind="ExternalInput")
    dxt = nc.dram_tensor("dxt", [NSTEP, IN_CH, B], f32, kind="ExternalInput")
    w1 = nc.dram_tensor("w1", [HID, MLP_W], f32, kind="ExternalInput")
    b1 = nc.dram_tensor("b1", [MLP_W, 1], f32, kind="ExternalInput")
    w2 = nc.dram_tensor("w2", [MLP_W, NBANK, 128], f32, kind="ExternalInput")
    b2 = nc.dram_tensor("b2", [128, NBANK], f32, kind="ExternalInput")
    wi = nc.dram_tensor("wi", [IN_CH, HID + 1], f32, kind="ExternalInput")
    smat = nc.dram_tensor("smat", [128, NBANK, HID + 1], f32,
                          kind="ExternalInput")
    outp = nc.dram_tensor("outp", [T, B], f32, kind="ExternalOutput")

    with tile.TileContext(nc) as tc, ExitStack() as ctx:
        const = ctx.enter_context(tc.tile_pool(name="const", bufs=1))
        zpool = ctx.enter_context(tc.tile_pool(name="zpool", bufs=2))
        hpool = ctx.enter_context(tc.tile_pool(name="hpool", bufs=2))
        fpool = ctx.enter_context(tc.tile_pool(name="fpool", bufs=2))
        gpool = ctx.enter_context(tc.tile_pool(name="gpool", bufs=3))
        dxpool = ctx.enter_context(tc.tile_pool(name="dxpool", bufs=5))
        psum_h = ctx.enter_context(tc.tile_pool(name="psum_h", bufs=2, space="PSUM"))
        psum_f = ctx.enter_context(tc.tile_pool(name="psum_f", bufs=2, space="PSUM"))
        psum_e = ctx.enter_context(tc.tile_pool(name="psum_e", bufs=2, space="PSUM"))

        w1_sb = const.tile([HID, MLP_W], f32)
        nc.sync.dma_start(w1_sb[:], w1[:])
        b1_sb = const.tile([MLP_W, 1], f32)
        nc.sync.dma_start(b1_sb[:], b1[:])
        w2_sb = const.tile([MLP_W, NBANK, 128], f32)
        nc.sync.dma_start(w2_sb[:], w2[:])
        b2_sb = const.tile([128, NBANK], f32)
        nc.sync.dma_start(b2_sb[:], b2[:])
        wi_sb = const.tile([IN_CH, HID + 1], f32)
        nc.sync.dma_start(wi_sb[:], wi[:])
        s_sb = const.tile([128, NBANK, HID + 1], f32)
        nc.sync.dma_start(s_sb[:], smat[:])
        s_sb_r = const.tile([128, NBANK, HID + 1], f32r, name="s_sb_r")
        nc.vector.tensor_copy(s_sb_r[:], s_sb[:])
        u0t_sb = const.tile([IN_CH, B], f32)
        nc.sync.dma_start(u0t_sb[:], u0t[:])

        # f32r alias of W2 for the late phase (PE rounds internally; the
        # DVE copy applies the same rounding, value-identical)
        w2_13 = const.tile([MLP_W, NBANK, 128], f32r, name="w2_13")
        nc.vector.tensor_copy(w2_13[:], w2_sb[:])

        z_sb = [None] * NCHAIN
        dx_tiles = {}
        g_banks = [None] * NBANK

        def init_chains():
            z0_ps = psum_e.tile([HID + 1, NCHAIN, Bc], f32, tag="e_ps",
                                name="z0_ps")
            for c in range(NCHAIN):
                cs = slice(c * Bc, (c + 1) * Bc)
                nc.tensor.matmul(
                    z0_ps[:, c, :], wi_sb[:], u0t_sb[:, cs],
                    start=True, stop=True
                )
                z_c = zpool.tile([HID + 1, Bc], f32, tag=f"z{c}", name=f"z_sb{c}")
                nc.vector.tensor_copy(z_c[:], z0_ps[:, c, :])
                nc.sync.dma_start(outp[0:1, cs], z_c[HID : HID + 1, :])
                z_sb[c] = z_c

        def frag_mm1_h(c, t, lo, h_tile):
            """fp32 mm1 per chain; relu+bias (DVE) writes this chain's half
            of the shared h tile."""
            h_ps = psum_h.tile([MLP_W, Bc], f32, tag="h_ps", name="h_ps")
            nc.tensor.matmul(
                h_ps[:], w1_sb[:], z_sb[c][0:HID, :], start=True, stop=True
            )
            # ScalarE: closer to PSUM, and its queue is idle here (the DVE
            # queue still holds the z adds that gate this step's mm1s)
            nc.scalar.activation(
                h_tile[:, c, :], h_ps[:], AF.Relu, bias=b1_sb[:, 0:1]
            )

        def frag_mm2_g(t, h_tile, lo):
            """per bank: both chains' matmuls into one PSUM tile, then ONE
            [128, 512] tanh (b2_j is per-bank, same for both chains) and
            ONE [128, 512] g multiply against the full dx tile."""
            dx_sb = dx_tiles[t]
            w2_use = w2_13 if lo else w2_sb
            for j in range(NBANK):
                f_ps = psum_f.tile([128, NCHAIN, Bc], f32, tag=f"f_ps{j}",
                                   bufs=1, name=f"f_ps{j}")
                nc.tensor.matmul(f_ps[:], w2_use[:, j, :], h_tile[:],
                                 start=True, stop=True)
                f_sb = fpool.tile([128, NCHAIN, Bc], f16 if lo else f32,
                                  tag=f"f_r{j}" if lo else f"f_f{j}",
                                  name=f"f_sb{j}")
                nc.scalar.activation(
                    f_sb[:], f_ps[:], AF.Tanh, bias=b2_sb[:, j : j + 1]
                )
                g_sb = gpool.tile([128, NCHAIN, Bc], f32r if lo else f32,
                                  tag=f"g_r{j}" if lo else f"g_f{j}",
                                  name=f"g_sb{j}")
                getattr(nc, kn["gmul"][j]).tensor_mul(g_sb[:], f_sb[:], dx_sb[:])
                g_banks[j] = g_sb

        def frag_red_both(t, lo):
            e_ps = psum_e.tile([HID + 1, NCHAIN, Bc], f32, tag="e_ps",
                               name="e_ps")
            s_use = s_sb_r if lo else s_sb
            for j in range(NBANK):
                nc.tensor.matmul(e_ps[:], s_use[:, j, :], g_banks[j][:],
                                 start=j == 0, stop=j == NBANK - 1)
            for c in range(NCHAIN):
                cs = slice(c * Bc, (c + 1) * Bc)
                z_prev = z_sb[c]
                z_sb[c] = zpool.tile([HID + 1, Bc], f32, tag=f"z{c}",
                                     name=f"z_sb{c}")
                getattr(nc, kn["add"]).tensor_add(
                    z_sb[c][:], e_ps[:, c, :], z_prev[:]
                )
                nc.sync.dma_start(outp[t + 1 : t + 2, cs],
                                  z_sb[c][HID : HID + 1, :])

        def dma_dx(t):
            if t >= NSTEP:
                return
            dx_sb = dxpool.tile([128, B], f32, tag="dx", name="dx_sb")
            nc.sync.dma_start(
                dx_sb[:], dxt[t][None, :, :].to_broadcast([128 // IN_CH, IN_CH, B])
            )
            dx_tiles[t] = dx_sb
            if t - DX_PREFETCH - 1 in dx_tiles:
                del dx_tiles[t - DX_PREFETCH - 1]

        def scan_body():
            init_chains()
            dx_tiles.clear()
            for t in range(DX_PREFETCH):
                dma_dx(t)
            for t in range(NSTEP):
                lo = t >= kn["t0"]
                dma_dx(t + DX_PREFETCH)
                h_tile = hpool.tile([MLP_W, NCHAIN, Bc], f32r if lo else f32,
                                    tag="h_r" if lo else "h_f", name="h_tile")
                frag_mm1_h(0, t, lo, h_tile)
                frag_mm1_h(1, t, lo, h_tile)
                frag_mm2_g(t, h_tile, lo)
                frag_red_both(t, lo)

        if repeat == 1:
            scan_body()
        else:
            # hardware loop: trip count is a runtime scalar, so timing
            # amplification costs no extra instructions
            with tc.For_i(0, repeat):
                scan_body()

    nc.compile()
    return nc


def _prep_host(u0, coeffs, W1, b1, W2, b2, Wi, bi, Wr, br):
    f32 = np.float32

    u0t_full = np.empty((IN_CH, B_FULL), f32)
    u0t_full[: IN_CH - 1] = u0.T
    u0t_full[IN_CH - 1] = 1.0

    dX = (coeffs[:, 1:] - coeffs[:, :-1]).astype(f32)  # [B_FULL, NSTEP, IN_CH]
    dxt_small = np.ascontiguousarray(dX.transpose(1, 2, 0))  # [NSTEP, 8, B_FULL]

    wi_mat = np.empty((IN_CH, HID + 1), f32)
    wi_mat[: IN_CH - 1, :HID] = Wi
    wi_mat[IN_CH - 1, :HID] = bi
    wi_mat[: IN_CH - 1, HID] = (Wi @ Wr)[:, 0]
    wi_mat[IN_CH - 1, HID] = float(bi @ Wr[:, 0] + br[0])

    w2_banks = np.ascontiguousarray(W2.reshape(MLP_W, NBANK, 128))
    b2_banks = np.ascontiguousarray(b2.reshape(NBANK, 128).T)

    p = np.arange(128)
    s_full = np.zeros((128, NBANK, HID + 1), f32)
    for j in range(NBANK):
        s_full[p, j, 16 * j + p // IN_CH] = 1.0
        s_full[p, j, HID] = Wr[16 * j + p // IN_CH, 0]

    return {
        "u0t": u0t_full,
        "dxt": dxt_small,
        "w1": np.ascontiguousarray(W1.astype(f32)),
        "b1": np.ascontiguousarray(b1.astype(f32).reshape(MLP_W, 1)),
        "w2": w2_banks.astype(f32),
        "b2": b2_banks.astype(f32),
        "wi": wi_mat,
        "smat": s_full,
    }


def _make_in_maps(full):
    in_maps = []
    for c in range(N_CORES):
        sl = slice(c * B, (c + 1) * B)
        in_maps.append(
            {
                "u0t": np.ascontiguousarray(full["u0t"][:, sl]),
                "dxt": np.ascontiguousarray(full["dxt"][:, :, sl]),
                "w1": full["w1"],
                "b1": full["b1"],
                "w2": full["w2"],
                "b2": full["b2"],
                "wi": full["wi"],
                "smat": full["smat"],
            }
        )
    return in_maps


def kernel(u0, coeffs, W1, b1, W2, b2, Wi, bi, Wr, br, repeat=None, knobs=None):
    from concourse.bass_utils import run_bass_kernel_spmd

    full = _prep_host(
        np.asarray(u0, np.float32), np.asarray(coeffs, np.float32),
        np.asarray(W1, np.float32), np.asarray(b1, np.float32),
        np.asarray(W2, np.float32), np.asarray(b2, np.float32),
        np.asarray(Wi, np.float32), np.asarray(bi, np.float32),
        np.asarray(Wr, np.float32).reshape(HID, OUT),
        np.asarray(br, np.float32).reshape(OUT),
    )
    in_maps = _make_in_maps(full)

    rep = REPEAT if repeat is None else repeat
    key = ("nc", rep, tuple(sorted(knobs.items())) if knobs else None)
    if key not in _CACHE:
        _CACHE[key] = _build_bass(rep, knobs)
    nc = _CACHE[key]

    res = run_bass_kernel_spmd(nc, in_maps, core_ids=list(range(N_CORES)))
    outs = res.results

    out_full = np.empty((B_FULL, T, OUT), np.float32)
    for c in range(N_CORES):
        out_full[c * B : (c + 1) * B, :, 0] = outs[c]["outp"].T
    return out_full
